# revision 19
# baseline (speedup 1.0000x reference)
"""Neighbor-slice attention (nn_AttentionModule) on 8 TRN2 NeuronCores.

Layout strategy (per core, 2 of 16 slices + 1 halo slice each side packed by
the host). All matmuls run in bf16 (2 cols/cycle on the PE, no 512-multiple
free-dim restriction, so hw=2304 is streamed unpadded):
  - features slice X[s]: SBUF f32 (residual) + bf16 copy (matmul input)
  - qT/kT projections:   (64 ci, hw) bf16 via matmul lhsT=[WqT|WkT], rhs=Xbf
  - v projection:        (hw, ci) chunks via lhsT=Xbf_chunk, rhs=WvT, stored
                         bf16 with a ones column appended -> y matmul yields
                         softmax denominators for free (row 64)
  - attention:           ft chunk (128 k, qw<=512) = k_chunk @ qT (bf16)
                         exp on ACT (or DVE Schraudolph fast-exp per
                         EXP_DVE_PRED) PSUM->SBUF bf16
                         yT_aug (65, qw) accumulated over 18 k-chunks
  - denom:               recip row via DVE reciprocal_approx_fast on (1, qw),
                         gpsimd broadcast to 64 partitions, folded into the
                         ysb evacuation multiply (DVE)
  - z:                   both sides accumulate into one PSUM tile; single
                         evac adds c2 + residual x (DVE scalar_tensor_tensor)
  - biases: bq/bk applied on PSUM evac; bv/bz folded into 2*(Wz@bv+bz)
"""

import sys

for _p in ("/opt/trn_rl_repo",):
    if _p not in sys.path:
        sys.path.insert(0, _p)

import numpy as np

N_FULL, C, H, W = 16, 128, 48, 48
HW = H * W            # 2304
CI = C // 2           # 64
KC = HW // 128        # 18 k-chunks per slice
NCORES = 8
NLOC = N_FULL // NCORES  # 2 local slices per core

# q-blocks (start, width); width <= 512 (one PSUM bank)
QBS = [(0, 512), (512, 512), (1024, 512), (1536, 512), (2048, 256)]

# Schraudolph fast-exp on DVE for a subset of (n, side, qi, j) units to
# offload the ACT engine.
EXP_DVE_PRED = lambda n, side, qi, j: (j % 2 == 1)

# bf16 Schraudolph constants: bits16 = round(x*log2(e)*128 + B16)
_S16 = 184.66496736235803          # 2**7 / ln(2)
_B16 = 16256.0 - 4.75              # 127*2**7 with mid-sawtooth correction

_NC_CACHE = {}
LAST_RESULTS = None
TRACE = False


def _build_nc():
    import concourse.bass as bass
    import concourse.mybir as mybir
    import concourse.tile as tile
    from concourse import bacc

    f32 = mybir.dt.float32
    bf16 = mybir.dt.bfloat16
    f16 = mybir.dt.float16
    i16 = mybir.dt.int16
    FT = mybir.ActivationFunctionType

    nc = bacc.Bacc()

    x4_d = nc.declare_dram_parameter("x4", [4, C, HW], f32, isOutput=False)
    wqk_d = nc.declare_dram_parameter("wqk", [C, C], f32, isOutput=False)
    wv_d = nc.declare_dram_parameter("wv", [C, CI], f32, isOutput=False)
    wz_d = nc.declare_dram_parameter("wz", [CI, C], f32, isOutput=False)
    bqk_d = nc.declare_dram_parameter("bqk", [C, 1], f32, isOutput=False)
    c2_d = nc.declare_dram_parameter("c2", [C, 1], f32, isOutput=False)
    out_d = nc.declare_dram_parameter("out", [NLOC, C, HW], f32, isOutput=True)

    with tile.TileContext(nc) as tc:
        with tc.tile_pool(name="const", bufs=1) as cpool, \
             tc.tile_pool(name="xt", bufs=4) as xpool, \
             tc.tile_pool(name="xb", bufs=4) as xbpool, \
             tc.tile_pool(name="qt", bufs=2) as qtpool, \
             tc.tile_pool(name="kt", bufs=4) as ktpool, \
             tc.tile_pool(name="vg", bufs=4) as vgpool, \
             tc.tile_pool(name="at", bufs=10) as atpool, \
             tc.tile_pool(name="ysb", bufs=8) as ypool, \
             tc.tile_pool(name="rb", bufs=4) as rbpool, \
             tc.tile_pool(name="osb", bufs=3) as opool:

            # ---- constants (f32 staging -> bf16 working copies) ----
            wqkf_t = cpool.tile([C, C], f32, tag="wqkf")
            wvf_t = cpool.tile([C, CI], f32, tag="wvf")
            wzf_t = cpool.tile([CI, C], f32, tag="wzf")
            wqk_t = cpool.tile([C, C], f16, tag="wqk")
            wv_t = cpool.tile([C, CI], f16, tag="wv")
            wz_t = cpool.tile([CI, C], bf16, tag="wz")
            bqk_t = cpool.tile([C, 1], f32, tag="bqk")
            c2_t = cpool.tile([C, 1], f32, tag="c2")

            nc.sync.dma_start(out=wqkf_t, in_=wqk_d[:, :])
            nc.sync.dma_start(out=wvf_t, in_=wv_d[:, :])
            nc.sync.dma_start(out=wzf_t, in_=wz_d[:, :])
            nc.sync.dma_start(out=bqk_t, in_=bqk_d[:, :])
            nc.sync.dma_start(out=c2_t, in_=c2_d[:, :])
            nc.vector.tensor_copy(wqk_t, wqkf_t)
            nc.vector.tensor_copy(wv_t, wvf_t)
            nc.vector.tensor_copy(wz_t, wzf_t)

            # ---- load features (f32 for residual) + f16 matmul copies ----
            # casts run on idle GPSIMD so ACT/DVE stay free for evacuations
            x_t = []
            xb_t = []
            for s in range(4):
                xt = xpool.tile([C, HW], f32, tag="xt")
                nc.sync.dma_start(out=xt, in_=x4_d[s])
                xb = xbpool.tile([C, HW], f16, tag="xb")
                nc.vector.tensor_copy(xb, xt)
                x_t.append(xt)
                xb_t.append(xb)

            # ---- projections (all f16); pq in thirds, double-buffered so
            # the PE never waits on the evacuations (qt on ACT, kt on DVE) --
            qt_t = [None, None]      # local slices only (x4 idx 1, 2)
            kt_t = [None] * 4
            vg_t = [None] * 4
            T3 = HW // 3             # 768
            with tc.tile_pool(name="pp", bufs=2, space="PSUM") as pp, \
                 tc.tile_pool(name="pv", bufs=1, space="PSUM") as pv:
                for s in range(4):
                    if s in (1, 2):
                        qt = qtpool.tile([CI, HW], f16, tag="qt")
                        qt_t[s - 1] = qt
                    kt = ktpool.tile([CI, HW], f16, tag="kt")
                    kt_t[s] = kt
                    for t in range(3):
                        t0 = T3 * t
                        pq = pp.tile([C, T3], f32, tag="pp")
                        for (b0, bw) in ((0, 512), (512, 256)):
                            nc.tensor.matmul(pq[:, b0:b0 + bw], lhsT=wqk_t,
                                             rhs=xb_t[s][:, t0 + b0:t0 + b0 + bw],
                                             start=True, stop=True)
                        if s in (1, 2):
                            nc.scalar.activation(qt_t[s - 1][:, t0:t0 + T3],
                                                 pq[0:CI, :], FT.Identity,
                                                 bias=bqk_t[0:CI, :])
                        nc.vector.tensor_scalar_add(kt[:, t0:t0 + T3],
                                                    pq[CI:C, :],
                                                    bqk_t[CI:C, :])

                    pvt = pv.tile([C, KC * CI], f32, tag="pv")
                    for j in range(KC):
                        nc.tensor.matmul(pvt[:, CI * j:CI * (j + 1)],
                                         lhsT=xb_t[s][:, 128 * j:128 * (j + 1)],
                                         rhs=wv_t, start=True, stop=True)
                    # ones column at 0 so the softmax denominator lands on
                    # yps partition 0 (custom-DVE reciprocal reads partition 0
                    # of the tile regardless of AP partition offset); v at
                    # 64..128 because a 64-partition read must start at 0/64
                    vg = vgpool.tile([C, KC, 2 * CI], bf16, tag="vg")
                    nc.scalar.activation(
                        vg[:, :, CI:2 * CI],
                        pvt.rearrange("p (j d) -> p j d", d=CI), FT.Copy)
                    nc.gpsimd.memset(vg[:, :, 0:1], 1.0)
                    nc.gpsimd.memset(vg[:, :, 1:CI], 0.0)
                    vg_t[s] = vg

            # ---- attention ----
            # Both sides (before/after) of each q-block run concurrently with
            # a one-j skew between f and y matmuls; per j the PE queue is
            #   f0_j, f1_j, y0_{j-1}, y1_{j-1}
            # which gives each exp 5 matmuls (~1.1us warm) of cover before its
            # y needs it.  The unit tail is engineered to never block the
            # strict-FIFO ACT/DVE queues (head-of-line blocking there starves
            # the PE and re-latches the HAM clock gate to 1.2 GHz):
            #   - at unit end: yps row-copies to SBUF (one on ACT, one on
            #     DVE), recips (DVE, data already available), then broadcast
            #     and the 1/denom multiply run entirely on idle GPSIMD
            #   - z matmuls flush into the NEXT unit's PE queue at j==4,
            #     output evac (DVE) + DMA at j==6
            with tc.tile_pool(name="pf", bufs=5, space="PSUM") as pf, \
                 tc.tile_pool(name="py", bufs=2, space="PSUM") as py, \
                 tc.tile_pool(name="pz", bufs=1, space="PSUM") as pz:
                pend_z = None
                pend_out = None
                eidx = 0
                for n in range(NLOC):
                    for qi, (q0, w) in enumerate(QBS):
                        zps = pz.tile([C, w], f32, tag="zps")
                        yps = [py.tile([2 * CI, w], f32, tag="yps",
                                       name=f"yps{s}")
                               for s in range(2)]
                        at_prev = [None, None]
                        for j in range(KC):
                            at_cur = [None, None]
                            for side in range(2):
                                kv = n + 2 * side
                                ft = pf.tile([C, w], f32, tag="ft")
                                nc.tensor.matmul(
                                    ft,
                                    lhsT=kt_t[kv][:, 128 * j:128 * (j + 1)],
                                    rhs=qt_t[n][:, q0:q0 + w],
                                    start=True, stop=True)
                                at = atpool.tile([C, w], bf16, tag="at")
                                if eidx % 9 < 4:
                                    nc.vector.tensor_scalar(
                                        at.bitcast(i16), ft, _S16, _B16,
                                        op0=mybir.AluOpType.mult,
                                        op1=mybir.AluOpType.add)
                                else:
                                    nc.scalar.activation(at, ft, FT.Exp)
                                eidx += 1
                                at_cur[side] = at
                            if j == 4 and pend_z is not None:
                                pend_z()
                                pend_z = None
                            if j == 6 and pend_out is not None:
                                pend_out()
                                pend_out = None
                            if j > 0:
                                for side in range(2):
                                    kv = n + 2 * side
                                    nc.tensor.matmul(
                                        yps[side],
                                        lhsT=vg_t[kv][:, j - 1, :],
                                        rhs=at_prev[side],
                                        start=(j == 1), stop=False)
                            at_prev = at_cur
                        for side in range(2):
                            kv = n + 2 * side
                            nc.tensor.matmul(yps[side],
                                             lhsT=vg_t[kv][:, KC - 1, :],
                                             rhs=at_prev[side],
                                             start=False, stop=True)
                        # --- unit tail, part 1 (no PE, no FIFO blocking) ---
                        ycop = []
                        for side in range(2):
                            yc = ypool.tile([CI, w], f32, tag="ysb",
                                            name=f"ycop{side}")
                            if side == 0:
                                nc.scalar.activation(yc, yps[0][CI:2 * CI, :],
                                                     FT.Copy)
                            else:
                                nc.vector.tensor_copy(yc,
                                                      yps[1][CI:2 * CI, :])
                            ycop.append(yc)
                        ysbs = []
                        for side in range(2):
                            rrow = rbpool.tile([1, w], f32, tag="rr")
                            nc.vector.reciprocal_approx_fast(
                                rrow, yps[side][0:1, :])
                            rb64 = rbpool.tile([CI, w], f32, tag="rb")
                            nc.gpsimd.partition_broadcast(rb64, rrow)
                            ysb = ypool.tile([CI, w], bf16, tag="ysb",
                                             name=f"ysb{side}")
                            nc.gpsimd.tensor_mul(ysb, ycop[side], rb64)
                            ysbs.append(ysb)

                        def make_z(zps=zps, ysbs=ysbs):
                            def flush_z():
                                for side in range(2):
                                    nc.tensor.matmul(zps, lhsT=wz_t,
                                                     rhs=ysbs[side],
                                                     start=(side == 0),
                                                     stop=(side == 1))
                            return flush_z

                        def make_out(zps=zps, n=n, q0=q0, w=w):
                            def flush_out():
                                osb = opool.tile([C, w], f32, tag="osb",
                                                 name="osb")
                                nc.vector.scalar_tensor_tensor(
                                    out=osb, in0=zps, scalar=c2_t,
                                    in1=x_t[n + 1][:, q0:q0 + w],
                                    op0=mybir.AluOpType.add,
                                    op1=mybir.AluOpType.add)
                                nc.sync.dma_start(out=out_d[n][:, q0:q0 + w],
                                                  in_=osb)
                            return flush_out

                        pend_z = make_z()
                        pend_out = make_out()
                if pend_z is not None:
                    pend_z()
                if pend_out is not None:
                    pend_out()

    nc.compile()
    return nc


def _get_nc():
    if "nc" not in _NC_CACHE:
        _NC_CACHE["nc"] = _build_nc()
    return _NC_CACHE["nc"]


def _host_inputs(features, Wq, bq, Wk, bk, Wv, bv, Wz, bz):
    X = np.ascontiguousarray(np.asarray(features, np.float32).reshape(N_FULL, C, HW))
    wqk = np.ascontiguousarray(np.concatenate([Wq.T, Wk.T], axis=1), np.float32)
    wv = np.ascontiguousarray(np.asarray(Wv).T, np.float32)
    wz = np.ascontiguousarray(np.asarray(Wz).T, np.float32)
    bqk = np.concatenate([bq, bk]).astype(np.float32).reshape(C, 1)
    c2 = (2.0 * (np.asarray(Wz) @ np.asarray(bv) + np.asarray(bz))).astype(
        np.float32).reshape(C, 1)
    in_maps = []
    for i in range(NCORES):
        idx = [max(2 * i - 1, 0), 2 * i, 2 * i + 1, min(2 * i + 2, N_FULL - 1)]
        in_maps.append({
            "x4": np.ascontiguousarray(X[idx]),
            "wqk": wqk, "wv": wv, "wz": wz, "bqk": bqk, "c2": c2,
        })
    return in_maps


def kernel(features, Wq, bq, Wk, bk, Wv, bv, Wz, bz):
    global LAST_RESULTS
    from concourse.bass_utils import run_bass_kernel_spmd

    nc = _get_nc()
    in_maps = _host_inputs(features, Wq, bq, Wk, bk, Wv, bv, Wz, bz)
    res = run_bass_kernel_spmd(nc, in_maps, core_ids=list(range(NCORES)),
                               trace=TRACE)
    LAST_RESULTS = res
    out = np.empty((N_FULL, C, H, W), np.float32)
    for i in range(NCORES):
        out[2 * i:2 * i + 2] = res.results[i]["out"].reshape(NLOC, C, H, W)
    return out
